# revision 2
# baseline (speedup 1.0000x reference)
"""Trainium2 Bass kernel: per-point 3x3 Gaussian covariance from quaternion + log_scale.

cov = R diag(exp(log_scale)) R^T with R built from the normalized quaternion.

v2 design (vs baseline):
- Host pre-shards into PLANAR fp16 component arrays (w,x,y,z,l0,l1,l2), each
  [P=128, R] per core; device never deinterleaves.  Host re-assembles the 6
  unique covariance entries (planar fp16 tiles) into the [N,3,3] fp32 output.
- All device intermediates fp16 -> every DVE tensor_tensor runs in 2x mode.
- H = (n2/2)*R is built from half-squares and raw pair products (no quaternion
  normalization divide); scale folds as sh_j = exp(ls_j/2) * (2/n2), cov = M M^T
  with M = H diag(sh).
- 1/(n2/2) via the custom-DVE reciprocal_approx_fast (fp32) -> no Ln, so all
  ScalarE activations (Square/Exp/Copy) live in one table set (no reloads).
- Work split across engines: ScalarE does squares/exps, GpSimd does part of the
  add tree, VectorE the rest.
"""

import os
import numpy as np

import concourse.bass as bass
import concourse.bacc as bacc
import concourse.mybir as mybir
from concourse.tile import TileContext
from concourse.bass_utils import run_bass_kernel_spmd

AF = mybir.ActivationFunctionType
FP32 = mybir.dt.float32
FP16 = mybir.dt.float16

N_CORES = 8
N_FULL = 4_000_000
P = 128
R = 3920                      # rows per partition per core; 128*3920*8 = 4,014,080 >= N
NPC = P * R                   # points per core (padded)
F = int(os.environ.get("KERNEL_F", "980"))       # points per partition per tile
GP_EXTRA = int(os.environ.get("GP_EXTRA", "6"))  # gram adds moved to GpSimd
WORK_BUFS = int(os.environ.get("WORK_BUFS", "1"))

SQRT_HALF = 0.7071067811865476

_built = {}


def _build():
    key = (F, GP_EXTRA, WORK_BUFS)
    if key in _built:
        return _built[key]

    nc = bacc.Bacc("TRN2", target_bir_lowering=False, debug=False, num_devices=N_CORES)
    comps = {}
    for name in ("w", "x", "y", "z", "l0", "l1", "l2"):
        comps[name] = nc.dram_tensor(name, [P, R], FP16, kind="ExternalInput")
    outs = {}
    for name in ("c00", "c01", "c02", "c11", "c12", "c22"):
        outs[name] = nc.dram_tensor(name, [P, R], FP16, kind="ExternalOutput")

    cv = {k: t.ap() for k, t in comps.items()}
    ov = {k: t.ap() for k, t in outs.items()}

    with TileContext(nc) as tc:
        with (
            tc.tile_pool(name="io", bufs=2) as io,
            tc.tile_pool(name="otp", bufs=2) as ot_pool,
            tc.tile_pool(name="wk", bufs=WORK_BUFS) as wk,
        ):
            t0 = 0
            while t0 < R:
                f = min(F, R - t0)
                _tile_body(nc, io, ot_pool, wk, cv, ov, t0, f)
                t0 += f

    nc.compile()
    _built[key] = nc
    return nc


def _tile_body(nc, io, ot_pool, wk, cv, ov, t0, f):
    def w16(tag):
        return wk.tile([P, f], FP16, tag=tag, name=f"{tag}_{t0}")

    def w32(tag):
        return wk.tile([P, f], FP32, tag=tag, name=f"{tag}_{t0}")

    # ---- DMA in: 7 planar fp16 tiles ------------------------------------
    it = {}
    for k in ("w", "x", "y", "z", "l0", "l1", "l2"):
        it[k] = io.tile([P, f], FP16, tag=k, name=f"{k}_{t0}")
        nc.sync.dma_start(out=it[k], in_=cv[k][:, t0:t0 + f])

    # ---- ScalarE: half-squares and exp(ls/2) ----------------------------
    hsq = {}
    for k in ("w", "x", "y", "z"):
        hsq[k] = w16("h" + k)
        nc.scalar.activation(hsq[k], it[k], AF.Square, scale=SQRT_HALF)
    e = []
    for j in range(3):
        ej = w16(f"e{j}")
        nc.scalar.activation(ej, it[f"l{j}"], AF.Exp, scale=0.5)
        e.append(ej)

    # ---- GpSimd: diagonal add tree --------------------------------------
    pa = w16("pa"); pb = w16("pb"); pc = w16("pc"); pd = w16("pd")
    nc.gpsimd.tensor_sub(pa, hsq["w"], hsq["z"])
    nc.gpsimd.tensor_sub(pb, hsq["x"], hsq["y"])
    nc.gpsimd.tensor_add(pc, hsq["w"], hsq["z"])
    nc.gpsimd.tensor_add(pd, hsq["x"], hsq["y"])
    h00 = w16("h00"); h11 = w16("h11"); h22 = w16("h22")
    nc.gpsimd.tensor_add(h00, pa, pb)
    nc.gpsimd.tensor_sub(h11, pa, pb)
    nc.gpsimd.tensor_sub(h22, pc, pd)

    # ---- VectorE: n2h (fp32), reciprocal, sh ----------------------------
    n2h = w32("n2h")
    nc.vector.tensor_add(n2h, pc, pd)          # n2/2, fp32 out
    inv32 = w32("inv32")
    nc.vector.reciprocal_approx_fast(out=inv32, in_=n2h)   # 2/n2
    inv = w16("inv")
    nc.scalar.copy(out=inv, in_=inv32)
    sh = []
    for j in range(3):
        shj = w16(f"sh{j}")
        nc.vector.tensor_mul(shj, e[j], inv)
        sh.append(shj)

    # ---- VectorE: pair products and off-diagonal H ----------------------
    pr = {}
    for (a, b) in (("x", "y"), ("x", "z"), ("y", "z"),
                   ("w", "x"), ("w", "y"), ("w", "z")):
        t = w16(f"p{a}{b}")
        nc.vector.tensor_mul(t, it[a], it[b])
        pr[a + b] = t
    h01 = w16("h01"); h10 = w16("h10"); h02 = w16("h02")
    h20 = w16("h20"); h12 = w16("h12"); h21 = w16("h21")
    nc.vector.tensor_sub(h01, pr["xy"], pr["wz"])
    nc.vector.tensor_add(h10, pr["xy"], pr["wz"])
    nc.vector.tensor_add(h02, pr["xz"], pr["wy"])
    nc.vector.tensor_sub(h20, pr["xz"], pr["wy"])
    nc.vector.tensor_sub(h12, pr["yz"], pr["wx"])
    nc.vector.tensor_add(h21, pr["yz"], pr["wx"])

    H = [[h00, h01, h02], [h10, h11, h12], [h20, h21, h22]]

    # ---- M = H diag(sh) --------------------------------------------------
    M = [[None] * 3 for _ in range(3)]
    for i in range(3):
        for j in range(3):
            M[i][j] = w16(f"m{i}{j}")
            nc.vector.tensor_mul(M[i][j], H[i][j], sh[j])

    # ---- ScalarE: squares of M for diagonal cov -------------------------
    msq = [[None] * 3 for _ in range(3)]
    for i in range(3):
        for j in range(3):
            msq[i][j] = w16(f"q{i}{j}")
            nc.scalar.activation(msq[i][j], M[i][j], AF.Square)

    # ---- gram: diag from msq, off-diag products + adds ------------------
    ot = {k: ot_pool.tile([P, f], FP16, tag=k, name=f"{k}_{t0}")
          for k in ("c00", "c01", "c02", "c11", "c12", "c22")}

    # interleave the 12 adds between engines; GP_EXTRA of them on gpsimd
    adders = []
    for n in range(12):
        if n < 12 - GP_EXTRA:
            adders.append(nc.vector)
        else:
            adders.append(nc.gpsimd)
    ai = [0]

    def addt(out, a, b):
        eng = adders[ai[0] % 12]
        ai[0] += 1
        eng.tensor_add(out, a, b)

    for i, k, key in ((0, 0, "c00"), (1, 1, "c11"), (2, 2, "c22")):
        s = w16(f"s{key}")
        addt(s, msq[i][0], msq[i][1])
        addt(ot[key], s, msq[i][2])
    for i, k, key in ((0, 1, "c01"), (0, 2, "c02"), (1, 2, "c12")):
        g0 = w16(f"g0{key}"); g1 = w16(f"g1{key}"); g2 = w16(f"g2{key}")
        nc.vector.tensor_mul(g0, M[i][0], M[k][0])
        nc.vector.tensor_mul(g1, M[i][1], M[k][1])
        nc.vector.tensor_mul(g2, M[i][2], M[k][2])
        s = w16(f"s{key}")
        addt(s, g0, g1)
        addt(ot[key], s, g2)

    for key in ("c00", "c01", "c02", "c11", "c12", "c22"):
        nc.sync.dma_start(out=ov[key][:, t0:t0 + f], in_=ot[key])


def _pad_and_shard(quaternion, log_scale):
    n = quaternion.shape[0]
    ntot = N_CORES * NPC
    q16 = np.empty((ntot, 4), np.float16)
    l16 = np.empty((ntot, 3), np.float16)
    q16[:n] = quaternion.astype(np.float16)
    l16[:n] = log_scale.astype(np.float16)
    q16[n:] = np.array([1, 0, 0, 0], np.float16)
    l16[n:] = 0
    in_maps = []
    for i in range(N_CORES):
        sl = slice(i * NPC, (i + 1) * NPC)
        m = {}
        for ci, k in enumerate(("w", "x", "y", "z")):
            m[k] = np.ascontiguousarray(q16[sl, ci]).reshape(P, R)
        for ci, k in enumerate(("l0", "l1", "l2")):
            m[k] = np.ascontiguousarray(l16[sl, ci]).reshape(P, R)
        in_maps.append(m)
    return in_maps


def kernel_with_stats(quaternion, log_scale, trace=False):
    quaternion = np.asarray(quaternion, dtype=np.float32)
    log_scale = np.asarray(log_scale, dtype=np.float32)
    n = quaternion.shape[0]
    nc = _build()
    in_maps = _pad_and_shard(quaternion, log_scale)
    res = run_bass_kernel_spmd(nc, in_maps, core_ids=list(range(N_CORES)), trace=trace)
    out = np.empty((n, 3, 3), np.float32)
    slots = {"c00": [(0, 0)], "c01": [(0, 1), (1, 0)], "c02": [(0, 2), (2, 0)],
             "c11": [(1, 1)], "c12": [(1, 2), (2, 1)], "c22": [(2, 2)]}
    for key, ps in slots.items():
        full = np.concatenate([r[key].reshape(-1) for r in res.results])[:n]
        full = full.astype(np.float32)
        for (i, k) in ps:
            out[:, i, k] = full
    return out, res


def kernel(quaternion, log_scale):
    out, _ = kernel_with_stats(quaternion, log_scale, trace=False)
    return out


# revision 5
# speedup vs baseline: 1.7132x; 1.7132x over previous
"""Trainium2 Bass kernel: per-point 3x3 Gaussian covariance from quaternion + log_scale.

cov = R diag(exp(log_scale)) R^T with R built from the normalized quaternion.

v2 design (vs baseline):
- Host pre-shards into PLANAR fp16 component arrays (w,x,y,z,l0,l1,l2), each
  [P=128, R] per core; device never deinterleaves.  Host re-assembles the 6
  unique covariance entries (planar fp16 tiles) into the [N,3,3] fp32 output.
- All device intermediates fp16 -> every DVE tensor_tensor runs in 2x mode.
- H = (n2/2)*R is built from half-squares and raw pair products (no quaternion
  normalization divide); scale folds as sh_j = exp(ls_j/2) * (2/n2), cov = M M^T
  with M = H diag(sh).
- 1/(n2/2) via the custom-DVE reciprocal_approx_fast (fp32) -> no Ln, so all
  ScalarE activations (Square/Exp/Copy) live in one table set (no reloads).
- Work split across engines: ScalarE does squares/exps, GpSimd does part of the
  add tree, VectorE the rest.
"""

import os
import numpy as np

import concourse.bass as bass
import concourse.bacc as bacc
import concourse.mybir as mybir
from concourse.tile import TileContext
from concourse.bass_utils import run_bass_kernel_spmd

AF = mybir.ActivationFunctionType
FP32 = mybir.dt.float32
FP16 = mybir.dt.float16

N_CORES = 8
N_FULL = 4_000_000
P = 128
R = 3920                      # rows per partition per core; 128*3920*8 = 4,014,080 >= N
NPC = P * R                   # points per core (padded)
F = int(os.environ.get("KERNEL_F", "980"))       # points per partition per tile
GP_EXTRA = int(os.environ.get("GP_EXTRA", "0"))  # ops on GpSimd (0: contention makes it a loss)
WORK_BUFS = int(os.environ.get("WORK_BUFS", "1"))

SQRT_HALF = 0.7071067811865476

_built = {}


def _build():
    key = (F, GP_EXTRA, WORK_BUFS)
    if key in _built:
        return _built[key]

    nc = bacc.Bacc("TRN2", target_bir_lowering=False, debug=False, num_devices=N_CORES)
    comps = {}
    for name in ("w", "x", "y", "z", "l0", "l1", "l2"):
        comps[name] = nc.dram_tensor(name, [P, R], FP16, kind="ExternalInput")
    outs = {}
    for name in ("c00", "c01", "c02", "c11", "c12", "c22"):
        outs[name] = nc.dram_tensor(name, [P, R], FP16, kind="ExternalOutput")

    cv = {k: t.ap() for k, t in comps.items()}
    ov = {k: t.ap() for k, t in outs.items()}

    with TileContext(nc) as tc:
        with (
            tc.tile_pool(name="io", bufs=2) as io,
            tc.tile_pool(name="otp", bufs=2) as ot_pool,
            tc.tile_pool(name="wk", bufs=WORK_BUFS) as wk,
        ):
            t0 = 0
            while t0 < R:
                f = min(F, R - t0)
                _tile_body(nc, io, ot_pool, wk, cv, ov, t0, f)
                t0 += f

    nc.compile()
    _built[key] = nc
    return nc


def _tile_body(nc, io, ot_pool, wk, cv, ov, t0, f):
    def w16(tag):
        return wk.tile([P, f], FP16, tag=tag, name=f"{tag}_{t0}")

    def w32(tag):
        return wk.tile([P, f], FP32, tag=tag, name=f"{tag}_{t0}")

    # ---- DMA in: 7 planar fp16 tiles ------------------------------------
    it = {}
    for k in ("w", "x", "y", "z", "l0", "l1", "l2"):
        it[k] = io.tile([P, f], FP16, tag=k, name=f"{k}_{t0}")
        nc.sync.dma_start(out=it[k], in_=cv[k][:, t0:t0 + f])

    # ---- ScalarE: half-squares and exp(ls/2) ----------------------------
    hsq = {}
    for k in ("w", "x", "y", "z"):
        hsq[k] = w16("h" + k)
        nc.scalar.activation(hsq[k], it[k], AF.Square, scale=SQRT_HALF)
    e = []
    for j in range(3):
        ej = w16(f"e{j}")
        nc.scalar.activation(ej, it[f"l{j}"], AF.Exp, scale=0.5)
        e.append(ej)

    # ---- diagonal add tree ----------------------------------------------
    pa = w16("pa"); pb = w16("pb"); pc = w16("pc"); pd = w16("pd")
    nc.vector.tensor_sub(pa, hsq["w"], hsq["z"])
    nc.vector.tensor_sub(pb, hsq["x"], hsq["y"])
    nc.vector.tensor_add(pc, hsq["w"], hsq["z"])
    nc.vector.tensor_add(pd, hsq["x"], hsq["y"])
    h00 = w16("h00"); h11 = w16("h11"); h22 = w16("h22")
    nc.vector.tensor_add(h00, pa, pb)
    nc.vector.tensor_sub(h11, pa, pb)
    nc.vector.tensor_sub(h22, pc, pd)

    # ---- VectorE: n2h (fp32), reciprocal, sh ----------------------------
    n2h = w32("n2h")
    nc.vector.tensor_add(n2h, pc, pd)          # n2/2, fp32 out
    inv32 = w32("inv32")
    nc.vector.reciprocal_approx_fast(out=inv32, in_=n2h)   # 2/n2
    inv = w16("inv")
    nc.scalar.copy(out=inv, in_=inv32)
    sh = []
    for j in range(3):
        shj = w16(f"sh{j}")
        nc.vector.tensor_mul(shj, e[j], inv)
        sh.append(shj)

    # ---- VectorE: pair products and off-diagonal H ----------------------
    pr = {}
    for (a, b) in (("x", "y"), ("x", "z"), ("y", "z"),
                   ("w", "x"), ("w", "y"), ("w", "z")):
        t = w16(f"p{a}{b}")
        nc.vector.tensor_mul(t, it[a], it[b])
        pr[a + b] = t
    h01 = w16("h01"); h10 = w16("h10"); h02 = w16("h02")
    h20 = w16("h20"); h12 = w16("h12"); h21 = w16("h21")
    nc.vector.tensor_sub(h01, pr["xy"], pr["wz"])
    nc.vector.tensor_add(h10, pr["xy"], pr["wz"])
    nc.vector.tensor_add(h02, pr["xz"], pr["wy"])
    nc.vector.tensor_sub(h20, pr["xz"], pr["wy"])
    nc.vector.tensor_sub(h12, pr["yz"], pr["wx"])
    nc.vector.tensor_add(h21, pr["yz"], pr["wx"])

    H = [[h00, h01, h02], [h10, h11, h12], [h20, h21, h22]]

    # ---- M = H diag(sh) --------------------------------------------------
    M = [[None] * 3 for _ in range(3)]
    for i in range(3):
        for j in range(3):
            M[i][j] = w16(f"m{i}{j}")
            nc.vector.tensor_mul(M[i][j], H[i][j], sh[j])

    # ---- ScalarE: squares of M for diagonal cov -------------------------
    msq = [[None] * 3 for _ in range(3)]
    for i in range(3):
        for j in range(3):
            msq[i][j] = w16(f"q{i}{j}")
            nc.scalar.activation(msq[i][j], M[i][j], AF.Square)

    # ---- gram: diag from msq, off-diag products + adds ------------------
    ot = {k: ot_pool.tile([P, f], FP16, tag=k, name=f"{k}_{t0}")
          for k in ("c00", "c01", "c02", "c11", "c12", "c22")}

    # off-diagonal first (pure DVE), diagonal last (waits on ScalarE msq)
    for i, k, key in ((0, 1, "c01"), (0, 2, "c02"), (1, 2, "c12")):
        g0 = w16(f"g0{key}"); g1 = w16(f"g1{key}"); g2 = w16(f"g2{key}")
        nc.vector.tensor_mul(g0, M[i][0], M[k][0])
        nc.vector.tensor_mul(g1, M[i][1], M[k][1])
        nc.vector.tensor_mul(g2, M[i][2], M[k][2])
        s = w16(f"s{key}")
        nc.vector.tensor_add(s, g0, g1)
        nc.vector.tensor_add(ot[key], s, g2)
    for i, k, key in ((0, 0, "c00"), (1, 1, "c11"), (2, 2, "c22")):
        s = w16(f"s{key}")
        nc.vector.tensor_add(s, msq[i][0], msq[i][1])
        nc.vector.tensor_add(ot[key], s, msq[i][2])

    for key in ("c00", "c01", "c02", "c11", "c12", "c22"):
        nc.sync.dma_start(out=ov[key][:, t0:t0 + f], in_=ot[key])


def _pad_and_shard(quaternion, log_scale):
    n = quaternion.shape[0]
    ntot = N_CORES * NPC
    q16 = np.empty((ntot, 4), np.float16)
    l16 = np.empty((ntot, 3), np.float16)
    q16[:n] = quaternion.astype(np.float16)
    l16[:n] = log_scale.astype(np.float16)
    q16[n:] = np.array([1, 0, 0, 0], np.float16)
    l16[n:] = 0
    in_maps = []
    for i in range(N_CORES):
        sl = slice(i * NPC, (i + 1) * NPC)
        m = {}
        for ci, k in enumerate(("w", "x", "y", "z")):
            m[k] = np.ascontiguousarray(q16[sl, ci]).reshape(P, R)
        for ci, k in enumerate(("l0", "l1", "l2")):
            m[k] = np.ascontiguousarray(l16[sl, ci]).reshape(P, R)
        in_maps.append(m)
    return in_maps


def kernel_with_stats(quaternion, log_scale, trace=False):
    quaternion = np.asarray(quaternion, dtype=np.float32)
    log_scale = np.asarray(log_scale, dtype=np.float32)
    n = quaternion.shape[0]
    nc = _build()
    in_maps = _pad_and_shard(quaternion, log_scale)
    res = run_bass_kernel_spmd(nc, in_maps, core_ids=list(range(N_CORES)), trace=trace)
    out = np.empty((n, 3, 3), np.float32)
    slots = {"c00": [(0, 0)], "c01": [(0, 1), (1, 0)], "c02": [(0, 2), (2, 0)],
             "c11": [(1, 1)], "c12": [(1, 2), (2, 1)], "c22": [(2, 2)]}
    for key, ps in slots.items():
        full = np.concatenate([r[key].reshape(-1) for r in res.results])[:n]
        full = full.astype(np.float32)
        for (i, k) in ps:
            out[:, i, k] = full
    return out, res


def kernel(quaternion, log_scale):
    out, _ = kernel_with_stats(quaternion, log_scale, trace=False)
    return out


# revision 6
# speedup vs baseline: 1.7349x; 1.0127x over previous
"""Trainium2 Bass kernel: per-point 3x3 Gaussian covariance from quaternion + log_scale.

cov = R diag(exp(log_scale)) R^T with R built from the normalized quaternion.

v4: fused-instruction design.
- Host uploads PLANAR fp16 component blocks concatenated per partition row:
  q_cat [P, 4, R] in component order (w, z, y, x), l_cat [P, 3, R].
  Host reassembles the 6 unique cov entries (out_cat [P, 6, R] fp16) into the
  [N,3,3] fp32 output.
- All intermediates fp16 -> DVE tensor_tensor runs in 2x mode.
- Concatenated work tiles + strided/broadcast access patterns fuse groups of
  identical elementwise ops into single wide DVE instructions (e.g. all 3
  columns of M = H diag(sh) in one op), cutting per-instruction fixed cost and
  semaphore traffic.
- ScalarE does the squares (of q and of M) and exps in 4 activations per tile,
  all from one activation table set; 1/(n2/2) via custom-DVE
  reciprocal_approx_fast (fp32).

Math: with half-squares hc = c^2/2 and H = (n2/2) R:
  pa = hw - hz, pb = hx - hy, pc = hw + hz, pd = hx + hy
  h00 = pa + pb, h11 = pa - pb, h22 = pc - pd, n2h = pc + pd
  h01 = xy - wz, h10 = xy + wz, h02 = xz + wy, h20 = xz - wy,
  h12 = yz - wx, h21 = yz + wx
  sh_j = exp(ls_j/2) / n2h;  M = H diag(sh);  cov = M M^T.
"""

import os
import numpy as np

import concourse.bass as bass
import concourse.bacc as bacc
import concourse.mybir as mybir
from concourse.tile import TileContext
from concourse.bass_utils import run_bass_kernel_spmd

AF = mybir.ActivationFunctionType
FP32 = mybir.dt.float32
FP16 = mybir.dt.float16

N_CORES = 8
N_FULL = 4_000_000
P = 128
R = 3920                      # rows per partition per core; 128*3920*8 = 4,014,080 >= N
NPC = P * R                   # points per core (padded)
F = int(os.environ.get("KERNEL_F", "980"))       # points per partition per tile
WORK_BUFS = int(os.environ.get("WORK_BUFS", "1"))
NO_BCAST = os.environ.get("NO_BCAST", "0") == "1"   # fallback: no stride-0 operands

SQRT_HALF = 0.7071067811865476

# q_cat component order
QW, QZ, QY, QX = 0, 1, 2, 3

_built = {}


def _build():
    key = (F, WORK_BUFS, NO_BCAST)
    if key in _built:
        return _built[key]

    nc = bacc.Bacc("TRN2", target_bir_lowering=False, debug=False, num_devices=N_CORES)
    q_cat = nc.dram_tensor("q_cat", [P, 4, R], FP16, kind="ExternalInput")
    l_cat = nc.dram_tensor("l_cat", [P, 3, R], FP16, kind="ExternalInput")
    o_cat = nc.dram_tensor("o_cat", [P, 6, R], FP16, kind="ExternalOutput")

    qv, lv, ov = q_cat.ap(), l_cat.ap(), o_cat.ap()

    with TileContext(nc) as tc:
        with (
            tc.tile_pool(name="io", bufs=2) as io,
            tc.tile_pool(name="otp", bufs=2) as ot_pool,
            tc.tile_pool(name="wk", bufs=WORK_BUFS) as wk,
        ):
            t0 = 0
            while t0 < R:
                f = min(F, R - t0)
                _tile_body(nc, io, ot_pool, wk, qv, lv, ov, t0, f)
                t0 += f

    nc.compile()
    _built[key] = nc
    return nc


def _tile_body(nc, io, ot_pool, wk, qv, lv, ov, t0, f):
    def wt(tag, units, dt=FP16):
        return wk.tile([P, units * f], dt, tag=tag, name=f"{tag}_{t0}")

    V = nc.vector

    # ---- DMA in ----------------------------------------------------------
    qt = io.tile([P, 4 * f], FP16, tag="q", name=f"q_{t0}")
    lt = io.tile([P, 3 * f], FP16, tag="l", name=f"l_{t0}")
    nc.sync.dma_start(out=qt.rearrange("p (k f) -> p k f", k=4),
                      in_=qv[:, :, t0:t0 + f])
    nc.sync.dma_start(out=lt.rearrange("p (k f) -> p k f", k=3),
                      in_=lv[:, :, t0:t0 + f])
    q3 = qt.rearrange("p (k f) -> p k f", k=4)

    # ---- ScalarE: half-squares (one act), exps (one act) -----------------
    hsq = wt("hsq", 4)
    nc.scalar.activation(hsq, qt, AF.Square, scale=SQRT_HALF)
    hs3 = hsq.rearrange("p (k f) -> p k f", k=4)   # (hw, hz, hy, hx)
    ec = wt("ec", 3)
    nc.scalar.activation(ec, lt, AF.Exp, scale=0.5)

    # ---- diagonal butterfly ---------------------------------------------
    # spd layout: [pa pb pc pd]
    spd = wt("spd", 4)
    sp3 = spd.rearrange("p (k f) -> p k f", k=4)
    in0 = hs3[:, 0:4:3, :]                # (hw, hx)
    in1 = hs3[:, 1:3, :]                  # (hz, hy)
    V.tensor_sub(sp3[:, 0:2, :], in0, in1)     # (pa, pb)
    V.tensor_add(sp3[:, 2:4, :], in0, in1)     # (pc, pd)
    # hdiag layout: [h00 h11 h22 n2h]
    hd = wt("hd", 4)
    hd3 = hd.rearrange("p (k f) -> p k f", k=4)
    i0 = sp3[:, 0:4:2, :]                 # (pa, pc)
    i1 = sp3[:, 1:4:2, :]                 # (pb, pd)
    V.tensor_sub(hd3[:, 1:3, :], i0, i1)       # (h11, h22)
    V.tensor_add(hd3[:, 0:4:3, :], i0, i1)     # (h00, n2h)

    # ---- reciprocal of n2h ----------------------------------------------
    n2h32 = wt("n2h32", 1, FP32)
    V.tensor_copy(n2h32, hd[:, 3 * f:4 * f])
    inv32 = wt("inv32", 1, FP32)
    V.reciprocal_approx_fast(out=inv32, in_=n2h32)
    inv = wt("inv", 1)
    nc.scalar.copy(out=inv, in_=inv32)

    # ---- sh = e * inv (broadcast) ---------------------------------------
    sh = wt("sh", 3)
    if NO_BCAST:
        for j in range(3):
            V.tensor_mul(sh[:, j * f:(j + 1) * f], ec[:, j * f:(j + 1) * f], inv)
    else:
        invb = inv.rearrange("p (k f) -> p k f", k=1).broadcast_to((P, 3, f))
        V.tensor_mul(sh.rearrange("p (k f) -> p k f", k=3),
                     ec.rearrange("p (k f) -> p k f", k=3), invb)

    # ---- pair products: p layout [pxy pxz pzy pwz pwy pwx] ---------------
    pp = wt("pp", 6)
    p6 = pp.rearrange("p (k f) -> p k f", k=6)
    if NO_BCAST:
        for j, qi in enumerate((QZ, QY, QX)):
            V.tensor_mul(p6[:, 3 + j, :], q3[:, QW, :], q3[:, qi, :])
    else:
        wb = q3[:, QW:QW + 1, :].broadcast_to((P, 3, f))
        V.tensor_mul(p6[:, 3:6, :], wb, q3[:, 1:4, :])   # (pwz, pwy, pwx)
    V.tensor_mul(p6[:, 0, :], q3[:, QX, :], q3[:, QY, :])  # pxy
    V.tensor_mul(p6[:, 1, :], q3[:, QX, :], q3[:, QZ, :])  # pxz
    V.tensor_mul(p6[:, 2, :], q3[:, QZ, :], q3[:, QY, :])  # pzy = yz

    # ---- off-diagonal H: ho layout [h01 h20 h12 | h10 h02 h21] ----------
    ho = wt("ho", 6)
    V.tensor_sub(ho[:, 0:3 * f], pp[:, 0:3 * f], pp[:, 3 * f:6 * f])
    V.tensor_add(ho[:, 3 * f:6 * f], pp[:, 0:3 * f], pp[:, 3 * f:6 * f])

    # ---- M = H diag(sh); M layout j-major: [M00 M10 M20 M01 M11 M21 M02 M12 M22]
    mm = wt("mm", 9)
    m9 = mm.rearrange("p (k f) -> p k f", k=9)
    # diagonal: (h00,h11,h22) * (sh0,sh1,sh2) -> slots 0,4,8
    V.tensor_mul(m9[:, 0:9:4, :],
                 hd3[:, 0:3, :],
                 sh.rearrange("p (k f) -> p k f", k=3))
    # off-diagonals: slot = 3j + i
    for src, (i, j) in ((0, (0, 1)), (1, (2, 0)), (2, (1, 2)),
                        (3, (1, 0)), (4, (0, 2)), (5, (2, 1))):
        V.tensor_mul(m9[:, 3 * j + i, :], ho[:, src * f:(src + 1) * f],
                     sh[:, j * f:(j + 1) * f])

    # ---- ScalarE: squares of M (one act) --------------------------------
    msq = wt("msq", 9)
    nc.scalar.activation(msq, mm, AF.Square)
    mq9 = msq.rearrange("p (k f) -> p k f", k=9)

    # ---- gram ------------------------------------------------------------
    ot = ot_pool.tile([P, 6 * f], FP16, tag="oc", name=f"oc_{t0}")
    o6 = ot.rearrange("p (k f) -> p k f", k=6)
    # off-diag products: g layout pair-major [(01)j0 j1 j2 | (02)... | (12)...]
    gg = wt("gg", 9)
    g9 = gg.rearrange("p (k f) -> p k f", k=9)
    for pi, (i, k) in enumerate(((0, 1), (0, 2), (1, 2))):
        V.tensor_mul(g9[:, 3 * pi:3 * (pi + 1), :],
                     m9[:, i:9:3, :], m9[:, k:9:3, :])
    s3 = wt("s3", 3)
    V.tensor_add(s3.rearrange("p (k f) -> p k f", k=3),
                 g9[:, 0:9:3, :], g9[:, 1:9:3, :])
    V.tensor_add(o6[:, 0:3, :],
                 s3.rearrange("p (k f) -> p k f", k=3), g9[:, 2:9:3, :])
    # diag: cov_ii = sum_j msq[3j+i]
    sd = wt("sd", 3)
    V.tensor_add(sd, msq[:, 0:3 * f], msq[:, 3 * f:6 * f])
    V.tensor_add(o6[:, 3:6, :],
                 sd.rearrange("p (k f) -> p k f", k=3), mq9[:, 6:9, :])

    nc.sync.dma_start(out=ov[:, :, t0:t0 + f], in_=o6)


def _pad_and_shard(quaternion, log_scale):
    n = quaternion.shape[0]
    ntot = N_CORES * NPC
    q16 = np.empty((ntot, 4), np.float16)
    l16 = np.empty((ntot, 3), np.float16)
    q16[:n] = quaternion.astype(np.float16)
    l16[:n] = log_scale.astype(np.float16)
    q16[n:] = np.array([1, 0, 0, 0], np.float16)
    l16[n:] = 0
    in_maps = []
    comp_order = (0, 3, 2, 1)   # (w, z, y, x) columns of quaternion
    for i in range(N_CORES):
        sl = slice(i * NPC, (i + 1) * NPC)
        qc = np.empty((P, 4, R), np.float16)
        for k, ci in enumerate(comp_order):
            qc[:, k, :] = np.ascontiguousarray(q16[sl, ci]).reshape(P, R)
        lc = np.empty((P, 3, R), np.float16)
        for ci in range(3):
            lc[:, ci, :] = np.ascontiguousarray(l16[sl, ci]).reshape(P, R)
        in_maps.append({"q_cat": qc, "l_cat": lc})
    return in_maps


def kernel_with_stats(quaternion, log_scale, trace=False):
    quaternion = np.asarray(quaternion, dtype=np.float32)
    log_scale = np.asarray(log_scale, dtype=np.float32)
    n = quaternion.shape[0]
    nc = _build()
    in_maps = _pad_and_shard(quaternion, log_scale)
    res = run_bass_kernel_spmd(nc, in_maps, core_ids=list(range(N_CORES)), trace=trace)
    out = np.empty((n, 3, 3), np.float32)
    # o_cat slots: [c01 c02 c12 | c00 c11 c22]
    slots = {0: [(0, 1), (1, 0)], 1: [(0, 2), (2, 0)], 2: [(1, 2), (2, 1)],
             3: [(0, 0)], 4: [(1, 1)], 5: [(2, 2)]}
    for s, ps in slots.items():
        full = np.concatenate([r["o_cat"][:, s, :].reshape(-1) for r in res.results])[:n]
        full = full.astype(np.float32)
        for (i, k) in ps:
            out[:, i, k] = full
    return out, res


def kernel(quaternion, log_scale):
    out, _ = kernel_with_stats(quaternion, log_scale, trace=False)
    return out


# revision 10
# speedup vs baseline: 1.7501x; 1.0088x over previous
"""Trainium2 Bass kernel: per-point 3x3 Gaussian covariance from quaternion + log_scale.

cov = R diag(exp(log_scale)) R^T with R built from the normalized quaternion.

v4: fused-instruction design.
- Host uploads PLANAR fp16 component blocks concatenated per partition row:
  q_cat [P, 4, R] in component order (w, z, y, x), l_cat [P, 3, R].
  Host reassembles the 6 unique cov entries (out_cat [P, 6, R] fp16) into the
  [N,3,3] fp32 output.
- All intermediates fp16 -> DVE tensor_tensor runs in 2x mode.
- Concatenated work tiles + strided/broadcast access patterns fuse groups of
  identical elementwise ops into single wide DVE instructions (e.g. all 3
  columns of M = H diag(sh) in one op), cutting per-instruction fixed cost and
  semaphore traffic.
- ScalarE does the squares (of q and of M) and exps in 4 activations per tile,
  all from one activation table set; 1/(n2/2) via custom-DVE
  reciprocal_approx_fast (fp32).

Math: with half-squares hc = c^2/2 and H = (n2/2) R:
  pa = hw - hz, pb = hx - hy, pc = hw + hz, pd = hx + hy
  h00 = pa + pb, h11 = pa - pb, h22 = pc - pd, n2h = pc + pd
  h01 = xy - wz, h10 = xy + wz, h02 = xz + wy, h20 = xz - wy,
  h12 = yz - wx, h21 = yz + wx
  sh_j = exp(ls_j/2) / n2h;  M = H diag(sh);  cov = M M^T.
"""

import os
import numpy as np

import concourse.bass as bass
import concourse.bacc as bacc
import concourse.mybir as mybir
from concourse.tile import TileContext
from concourse.bass_utils import run_bass_kernel_spmd

AF = mybir.ActivationFunctionType
FP32 = mybir.dt.float32
FP16 = mybir.dt.float16

N_CORES = 8
N_FULL = 4_000_000
P = 128
R = 3920                      # rows per partition per core; 128*3920*8 = 4,014,080 >= N
NPC = P * R                   # points per core (padded)
F = int(os.environ.get("KERNEL_F", "980"))       # points per partition per tile
WORK_BUFS = int(os.environ.get("WORK_BUFS", "1"))
NO_BCAST = os.environ.get("NO_BCAST", "0") == "1"   # fallback: no stride-0 operands

SQRT_HALF = 0.7071067811865476

# q_cat component order
QW, QZ, QY, QX = 0, 1, 2, 3

_built = {}


def _build():
    key = (F, WORK_BUFS, NO_BCAST)
    if key in _built:
        return _built[key]

    nc = bacc.Bacc("TRN2", target_bir_lowering=False, debug=False, num_devices=N_CORES)
    q_cat = nc.dram_tensor("q_cat", [P, 4, R], FP16, kind="ExternalInput")
    l_cat = nc.dram_tensor("l_cat", [P, 3, R], FP16, kind="ExternalInput")
    o_cat = nc.dram_tensor("o_cat", [P, 6, R], FP16, kind="ExternalOutput")

    qv, lv, ov = q_cat.ap(), l_cat.ap(), o_cat.ap()

    with TileContext(nc) as tc:
        with (
            tc.tile_pool(name="io", bufs=2) as io,
            tc.tile_pool(name="otp", bufs=2) as ot_pool,
            tc.tile_pool(name="wk", bufs=WORK_BUFS) as wk,
        ):
            t0 = 0
            while t0 < R:
                f = min(F, R - t0)
                _tile_body(nc, io, ot_pool, wk, qv, lv, ov, t0, f)
                t0 += f

    nc.compile()
    _built[key] = nc
    return nc


def _tile_body(nc, io, ot_pool, wk, qv, lv, ov, t0, f):
    def wt(tag, units, dt=FP16):
        return wk.tile([P, units * f], dt, tag=tag, name=f"{tag}_{t0}")

    V = nc.vector

    # ---- DMA in ----------------------------------------------------------
    qt = io.tile([P, 4 * f], FP16, tag="q", name=f"q_{t0}")
    lt = io.tile([P, 3 * f], FP16, tag="l", name=f"l_{t0}")
    nc.sync.dma_start(out=qt.rearrange("p (k f) -> p k f", k=4),
                      in_=qv[:, :, t0:t0 + f])
    nc.sync.dma_start(out=lt.rearrange("p (k f) -> p k f", k=3),
                      in_=lv[:, :, t0:t0 + f])
    q3 = qt.rearrange("p (k f) -> p k f", k=4)

    # ---- ScalarE: half-squares (one act), exps (one act) -----------------
    hsq = wt("hsq", 4)
    nc.scalar.activation(hsq, qt, AF.Square, scale=SQRT_HALF)
    hs3 = hsq.rearrange("p (k f) -> p k f", k=4)   # (hw, hz, hy, hx)
    ec = wt("ec", 3)
    nc.scalar.activation(ec, lt, AF.Exp, scale=0.5)

    # ---- diagonal butterfly ---------------------------------------------
    # spd layout: [pa pb pc pd]
    spd = wt("spd", 4)
    sp3 = spd.rearrange("p (k f) -> p k f", k=4)
    in0 = hs3[:, 0:4:3, :]                # (hw, hx)
    in1 = hs3[:, 1:3, :]                  # (hz, hy)
    V.tensor_sub(sp3[:, 0:2, :], in0, in1)     # (pa, pb)
    V.tensor_add(sp3[:, 2:4, :], in0, in1)     # (pc, pd)
    # hdiag layout: [h00 h11 h22 n2h]
    hd = wt("hd", 4)
    hd3 = hd.rearrange("p (k f) -> p k f", k=4)
    i0 = sp3[:, 0:4:2, :]                 # (pa, pc)
    i1 = sp3[:, 1:4:2, :]                 # (pb, pd)
    V.tensor_sub(hd3[:, 1:3, :], i0, i1)       # (h11, h22)
    V.tensor_add(hd3[:, 0:4:3, :], i0, i1)     # (h00, n2h)

    # ---- reciprocal of n2h ----------------------------------------------
    n2h32 = wt("n2h32", 1, FP32)
    nc.scalar.copy(out=n2h32, in_=hd[:, 3 * f:4 * f])
    inv32 = wt("inv32", 1, FP32)
    V.reciprocal_approx_fast(out=inv32, in_=n2h32)
    inv = wt("inv", 1)
    nc.scalar.copy(out=inv, in_=inv32)

    # ---- sh = e * inv (broadcast) ---------------------------------------
    sh = wt("sh", 3)
    if NO_BCAST:
        for j in range(3):
            V.tensor_mul(sh[:, j * f:(j + 1) * f], ec[:, j * f:(j + 1) * f], inv)
    else:
        invb = inv.rearrange("p (k f) -> p k f", k=1).broadcast_to((P, 3, f))
        V.tensor_mul(sh.rearrange("p (k f) -> p k f", k=3),
                     ec.rearrange("p (k f) -> p k f", k=3), invb)

    # ---- pair products: p layout [pxy pxz pzy pwz pwy pwx] ---------------
    pp = wt("pp", 6)
    p6 = pp.rearrange("p (k f) -> p k f", k=6)
    if NO_BCAST:
        for j, qi in enumerate((QZ, QY, QX)):
            V.tensor_mul(p6[:, 3 + j, :], q3[:, QW, :], q3[:, qi, :])
    else:
        wb = q3[:, QW:QW + 1, :].broadcast_to((P, 3, f))
        V.tensor_mul(p6[:, 3:6, :], wb, q3[:, 1:4, :])   # (pwz, pwy, pwx)
    V.tensor_mul(p6[:, 0, :], q3[:, QX, :], q3[:, QY, :])  # pxy
    V.tensor_mul(p6[:, 1, :], q3[:, QX, :], q3[:, QZ, :])  # pxz
    V.tensor_mul(p6[:, 2, :], q3[:, QZ, :], q3[:, QY, :])  # pzy = yz

    # ---- off-diagonal H: ho layout [h01 h20 h12 | h10 h02 h21] ----------
    ho = wt("ho", 6)
    V.tensor_sub(ho[:, 0:3 * f], pp[:, 0:3 * f], pp[:, 3 * f:6 * f])
    V.tensor_add(ho[:, 3 * f:6 * f], pp[:, 0:3 * f], pp[:, 3 * f:6 * f])

    # ---- M = H diag(sh); M layout j-major: [M00 M10 M20 M01 M11 M21 M02 M12 M22]
    mm = wt("mm", 9)
    m9 = mm.rearrange("p (k f) -> p k f", k=9)
    # diagonal: (h00,h11,h22) * (sh0,sh1,sh2) -> slots 0,4,8
    sh3 = sh.rearrange("p (k f) -> p k f", k=3)
    V.tensor_mul(m9[:, 0:9:4, :], hd3[:, 0:3, :], sh3)
    # off-diagonals, pair-fused: ho=[h01 h20 h12 h10 h02 h21], slot = 3j+i
    ho6 = ho.rearrange("p (k f) -> p k f", k=6)
    # (h01,h12)*(sh1,sh2) -> M(3,7); (h20,h21)*(sh0,sh1) -> M(2,5);
    # (h10,h02)*(sh0,sh2) -> M(1,6)
    V.tensor_mul(m9[:, 3:8:4, :], ho6[:, 0:3:2, :], sh3[:, 1:3, :])
    V.tensor_mul(m9[:, 2:6:3, :], ho6[:, 1:6:4, :], sh3[:, 0:2, :])
    V.tensor_mul(m9[:, 1:7:5, :], ho6[:, 3:5, :], sh3[:, 0:3:2, :])

    # ---- ScalarE: squares of M (one act) --------------------------------
    msq = wt("msq", 9)
    nc.scalar.activation(msq, mm, AF.Square)
    mq9 = msq.rearrange("p (k f) -> p k f", k=9)

    # ---- gram ------------------------------------------------------------
    ot = ot_pool.tile([P, 6 * f], FP16, tag="oc", name=f"oc_{t0}")
    o6 = ot.rearrange("p (k f) -> p k f", k=6)
    # off-diag products: g layout pair-major [(01)j0 j1 j2 | (02)... | (12)...]
    gg = wt("gg", 9)
    g9 = gg.rearrange("p (k f) -> p k f", k=9)
    if NO_BCAST:
        for pi, (i, k) in enumerate(((0, 1), (0, 2), (1, 2))):
            V.tensor_mul(g9[:, 3 * pi:3 * (pi + 1), :],
                         m9[:, i:9:3, :], m9[:, k:9:3, :])
    else:
        # pairs (0,1) and (0,2) share row0: one 6f op with a broadcast row0
        mij = mm.rearrange("p (j i f) -> p i j f", j=3, i=3)
        row0b = mij[:, 0:1, :, :].broadcast_to((P, 2, 3, f))
        g01_02 = gg[:, 0:6 * f].rearrange("p (q j f) -> p q j f", q=2, j=3)
        V.tensor_mul(g01_02, row0b, mij[:, 1:3, :, :])
        V.tensor_mul(g9[:, 6:9, :], m9[:, 1:9:3, :], m9[:, 2:9:3, :])
    s3 = wt("s3", 3)
    V.tensor_add(s3.rearrange("p (k f) -> p k f", k=3),
                 g9[:, 0:9:3, :], g9[:, 1:9:3, :])
    V.tensor_add(o6[:, 0:3, :],
                 s3.rearrange("p (k f) -> p k f", k=3), g9[:, 2:9:3, :])
    # diag: cov_ii = sum_j msq[3j+i]
    sd = wt("sd", 3)
    V.tensor_add(sd, msq[:, 0:3 * f], msq[:, 3 * f:6 * f])
    V.tensor_add(o6[:, 3:6, :],
                 sd.rearrange("p (k f) -> p k f", k=3), mq9[:, 6:9, :])

    nc.sync.dma_start(out=ov[:, :, t0:t0 + f], in_=o6)


def _pad_and_shard(quaternion, log_scale):
    n = quaternion.shape[0]
    ntot = N_CORES * NPC
    q16 = np.empty((ntot, 4), np.float16)
    l16 = np.empty((ntot, 3), np.float16)
    q16[:n] = quaternion.astype(np.float16)
    l16[:n] = log_scale.astype(np.float16)
    q16[n:] = np.array([1, 0, 0, 0], np.float16)
    l16[n:] = 0
    in_maps = []
    comp_order = (0, 3, 2, 1)   # (w, z, y, x) columns of quaternion
    for i in range(N_CORES):
        sl = slice(i * NPC, (i + 1) * NPC)
        qc = np.empty((P, 4, R), np.float16)
        for k, ci in enumerate(comp_order):
            qc[:, k, :] = np.ascontiguousarray(q16[sl, ci]).reshape(P, R)
        lc = np.empty((P, 3, R), np.float16)
        for ci in range(3):
            lc[:, ci, :] = np.ascontiguousarray(l16[sl, ci]).reshape(P, R)
        in_maps.append({"q_cat": qc, "l_cat": lc})
    return in_maps


def kernel_with_stats(quaternion, log_scale, trace=False):
    quaternion = np.asarray(quaternion, dtype=np.float32)
    log_scale = np.asarray(log_scale, dtype=np.float32)
    n = quaternion.shape[0]
    nc = _build()
    in_maps = _pad_and_shard(quaternion, log_scale)
    res = run_bass_kernel_spmd(nc, in_maps, core_ids=list(range(N_CORES)), trace=trace)
    out = np.empty((n, 3, 3), np.float32)
    # o_cat slots: [c01 c02 c12 | c00 c11 c22]
    slots = {0: [(0, 1), (1, 0)], 1: [(0, 2), (2, 0)], 2: [(1, 2), (2, 1)],
             3: [(0, 0)], 4: [(1, 1)], 5: [(2, 2)]}
    for s, ps in slots.items():
        full = np.concatenate([r["o_cat"][:, s, :].reshape(-1) for r in res.results])[:n]
        full = full.astype(np.float32)
        for (i, k) in ps:
            out[:, i, k] = full
    return out, res


def kernel(quaternion, log_scale):
    out, _ = kernel_with_stats(quaternion, log_scale, trace=False)
    return out


# revision 15
# speedup vs baseline: 1.8056x; 1.0317x over previous
"""Trainium2 Bass kernel: per-point 3x3 Gaussian covariance from quaternion + log_scale.

cov = R diag(exp(log_scale)) R^T with R built from the normalized quaternion.

v4: fused-instruction design.
- Host uploads PLANAR fp16 component blocks concatenated per partition row:
  q_cat [P, 4, R] in component order (w, z, y, x), l_cat [P, 3, R].
  Host reassembles the 6 unique cov entries (out_cat [P, 6, R] fp16) into the
  [N,3,3] fp32 output.
- All intermediates fp16 -> DVE tensor_tensor runs in 2x mode.
- Concatenated work tiles + strided/broadcast access patterns fuse groups of
  identical elementwise ops into single wide DVE instructions (e.g. all 3
  columns of M = H diag(sh) in one op), cutting per-instruction fixed cost and
  semaphore traffic.
- ScalarE does the squares (of q and of M) and exps in 4 activations per tile,
  all from one activation table set; 1/(n2/2) via custom-DVE
  reciprocal_approx_fast (fp32).

Math: with half-squares hc = c^2/2 and H = (n2/2) R:
  pa = hw - hz, pb = hx - hy, pc = hw + hz, pd = hx + hy
  h00 = pa + pb, h11 = pa - pb, h22 = pc - pd, n2h = pc + pd
  h01 = xy - wz, h10 = xy + wz, h02 = xz + wy, h20 = xz - wy,
  h12 = yz - wx, h21 = yz + wx
  sh_j = exp(ls_j/2) / n2h;  M = H diag(sh);  cov = M M^T.
"""

import os
import numpy as np

import concourse.bass as bass
import concourse.bacc as bacc
import concourse.mybir as mybir
from concourse.tile import TileContext
from concourse.bass_utils import run_bass_kernel_spmd

AF = mybir.ActivationFunctionType
FP32 = mybir.dt.float32
FP16 = mybir.dt.float16

N_CORES = 8
N_FULL = 4_000_000
P = 128
R = 3920                      # rows per partition per core; 128*3920*8 = 4,014,080 >= N
NPC = P * R                   # points per core (padded)
F = int(os.environ.get("KERNEL_F", "980"))       # points per partition per tile
TILES = os.environ.get("KERNEL_TILES", "980,980,980,700,280")  # overrides F if set
WORK_BUFS = int(os.environ.get("WORK_BUFS", "1"))
NO_BCAST = os.environ.get("NO_BCAST", "0") == "1"   # fallback: no stride-0 operands

SQRT_HALF = 0.7071067811865476

# q_cat component order
QW, QZ, QY, QX = 0, 1, 2, 3

_built = {}


def _tile_schedule():
    if TILES:
        fs = [int(x) for x in TILES.split(",")]
        assert sum(fs) == R, (fs, R)
        return fs
    fs = []
    t0 = 0
    while t0 < R:
        fs.append(min(F, R - t0))
        t0 += fs[-1]
    return fs


def _build():
    key = (F, TILES, WORK_BUFS, NO_BCAST)
    if key in _built:
        return _built[key]

    nc = bacc.Bacc("TRN2", target_bir_lowering=False, debug=False, num_devices=N_CORES)
    q_cat = nc.dram_tensor("q_cat", [P, 4, R], FP16, kind="ExternalInput")
    l_cat = nc.dram_tensor("l_cat", [P, 3, R], FP16, kind="ExternalInput")
    o_cat = nc.dram_tensor("o_cat", [P, 6, R], FP16, kind="ExternalOutput")

    qv, lv, ov = q_cat.ap(), l_cat.ap(), o_cat.ap()

    with TileContext(nc) as tc:
        with (
            tc.tile_pool(name="io", bufs=2) as io,
            tc.tile_pool(name="otp", bufs=2) as ot_pool,
            tc.tile_pool(name="wk", bufs=WORK_BUFS) as wk,
        ):
            t0 = 0
            for f in _tile_schedule():
                _tile_body(nc, io, ot_pool, wk, qv, lv, ov, t0, f)
                t0 += f

    nc.compile()
    _built[key] = nc
    return nc


def _tile_body(nc, io, ot_pool, wk, qv, lv, ov, t0, f):
    def wt(tag, units, dt=FP16):
        return wk.tile([P, units * f], dt, tag=tag, name=f"{tag}_{t0}")

    V = nc.vector

    # ---- DMA in ----------------------------------------------------------
    qt = io.tile([P, 4 * f], FP16, tag="q", name=f"q_{t0}")
    lt = io.tile([P, 3 * f], FP16, tag="l", name=f"l_{t0}")
    nc.sync.dma_start(out=qt.rearrange("p (k f) -> p k f", k=4),
                      in_=qv[:, :, t0:t0 + f])
    nc.sync.dma_start(out=lt.rearrange("p (k f) -> p k f", k=3),
                      in_=lv[:, :, t0:t0 + f])
    q3 = qt.rearrange("p (k f) -> p k f", k=4)

    # ---- ScalarE: half-squares (one act), exps (one act) -----------------
    hsq = wt("hsq", 4)
    nc.scalar.activation(hsq, qt, AF.Square, scale=SQRT_HALF)
    hs3 = hsq.rearrange("p (k f) -> p k f", k=4)   # (hw, hz, hy, hx)
    ec = wt("ec", 3)
    nc.scalar.activation(ec, lt, AF.Exp, scale=0.5)

    # ---- pair products first: DVE needs only the q DMA -------------------
    # p layout [pxy pxz pzy pwz pwy pwx]
    pp = wt("pp", 6)
    p6 = pp.rearrange("p (k f) -> p k f", k=6)
    if NO_BCAST:
        for j, qi in enumerate((QZ, QY, QX)):
            V.tensor_mul(p6[:, 3 + j, :], q3[:, QW, :], q3[:, qi, :])
    else:
        wb = q3[:, QW:QW + 1, :].broadcast_to((P, 3, f))
        V.tensor_mul(p6[:, 3:6, :], wb, q3[:, 1:4, :])   # (pwz, pwy, pwx)
    V.tensor_mul(p6[:, 0, :], q3[:, QX, :], q3[:, QY, :])  # pxy
    V.tensor_mul(p6[:, 1, :], q3[:, QX, :], q3[:, QZ, :])  # pxz
    V.tensor_mul(p6[:, 2, :], q3[:, QZ, :], q3[:, QY, :])  # pzy = yz

    # ---- diagonal butterfly ---------------------------------------------
    # spd layout: [pa pb pc pd]
    spd = wt("spd", 4)
    sp3 = spd.rearrange("p (k f) -> p k f", k=4)
    in0 = hs3[:, 0:4:3, :]                # (hw, hx)
    in1 = hs3[:, 1:3, :]                  # (hz, hy)
    V.tensor_sub(sp3[:, 0:2, :], in0, in1)     # (pa, pb)
    V.tensor_add(sp3[:, 2:4, :], in0, in1)     # (pc, pd)
    # hdiag layout: [h00 h11 h22 n2h]
    hd = wt("hd", 4)
    hd3 = hd.rearrange("p (k f) -> p k f", k=4)
    i0 = sp3[:, 0:4:2, :]                 # (pa, pc)
    i1 = sp3[:, 1:4:2, :]                 # (pb, pd)
    V.tensor_sub(hd3[:, 1:3, :], i0, i1)       # (h11, h22)
    V.tensor_add(hd3[:, 0:4:3, :], i0, i1)     # (h00, n2h)

    # ---- reciprocal of n2h ----------------------------------------------
    n2h32 = wt("n2h32", 1, FP32)
    nc.scalar.copy(out=n2h32, in_=hd[:, 3 * f:4 * f])
    inv32 = wt("inv32", 1, FP32)
    V.reciprocal_approx_fast(out=inv32, in_=n2h32)
    inv = wt("inv", 1)
    nc.scalar.copy(out=inv, in_=inv32)

    # ---- off-diagonal H (overlaps the ScalarE inv copy) -----------------
    # ho layout [h01 h20 h12 | h10 h02 h21]
    ho = wt("ho", 6)
    V.tensor_sub(ho[:, 0:3 * f], pp[:, 0:3 * f], pp[:, 3 * f:6 * f])
    V.tensor_add(ho[:, 3 * f:6 * f], pp[:, 0:3 * f], pp[:, 3 * f:6 * f])

    # ---- sh = e * inv (broadcast) ---------------------------------------
    sh = wt("sh", 3)
    if NO_BCAST:
        for j in range(3):
            V.tensor_mul(sh[:, j * f:(j + 1) * f], ec[:, j * f:(j + 1) * f], inv)
    else:
        invb = inv.rearrange("p (k f) -> p k f", k=1).broadcast_to((P, 3, f))
        V.tensor_mul(sh.rearrange("p (k f) -> p k f", k=3),
                     ec.rearrange("p (k f) -> p k f", k=3), invb)

    # ---- M = H diag(sh); M layout j-major: [M00 M10 M20 M01 M11 M21 M02 M12 M22]
    mm = wt("mm", 9)
    m9 = mm.rearrange("p (k f) -> p k f", k=9)
    # diagonal: (h00,h11,h22) * (sh0,sh1,sh2) -> slots 0,4,8
    sh3 = sh.rearrange("p (k f) -> p k f", k=3)
    V.tensor_mul(m9[:, 0:9:4, :], hd3[:, 0:3, :], sh3)
    # off-diagonals, pair-fused: ho=[h01 h20 h12 h10 h02 h21], slot = 3j+i
    ho6 = ho.rearrange("p (k f) -> p k f", k=6)
    # (h01,h12)*(sh1,sh2) -> M(3,7); (h20,h21)*(sh0,sh1) -> M(2,5);
    # (h10,h02)*(sh0,sh2) -> M(1,6)
    V.tensor_mul(m9[:, 3:8:4, :], ho6[:, 0:3:2, :], sh3[:, 1:3, :])
    V.tensor_mul(m9[:, 2:6:3, :], ho6[:, 1:6:4, :], sh3[:, 0:2, :])
    V.tensor_mul(m9[:, 1:7:5, :], ho6[:, 3:5, :], sh3[:, 0:3:2, :])

    # ---- ScalarE: squares of M (one act) --------------------------------
    msq = wt("msq", 9)
    nc.scalar.activation(msq, mm, AF.Square)
    mq9 = msq.rearrange("p (k f) -> p k f", k=9)

    # ---- gram ------------------------------------------------------------
    ot = ot_pool.tile([P, 6 * f], FP16, tag="oc", name=f"oc_{t0}")
    o6 = ot.rearrange("p (k f) -> p k f", k=6)
    # off-diag products: g layout pair-major [(01)j0 j1 j2 | (02)... | (12)...]
    gg = wt("gg", 9)
    g9 = gg.rearrange("p (k f) -> p k f", k=9)
    if NO_BCAST:
        for pi, (i, k) in enumerate(((0, 1), (0, 2), (1, 2))):
            V.tensor_mul(g9[:, 3 * pi:3 * (pi + 1), :],
                         m9[:, i:9:3, :], m9[:, k:9:3, :])
    else:
        # pairs (0,1) and (0,2) share row0: one 6f op with a broadcast row0
        mij = mm.rearrange("p (j i f) -> p i j f", j=3, i=3)
        row0b = mij[:, 0:1, :, :].broadcast_to((P, 2, 3, f))
        g01_02 = gg[:, 0:6 * f].rearrange("p (q j f) -> p q j f", q=2, j=3)
        V.tensor_mul(g01_02, row0b, mij[:, 1:3, :, :])
        V.tensor_mul(g9[:, 6:9, :], m9[:, 1:9:3, :], m9[:, 2:9:3, :])
    s3 = wt("s3", 3)
    V.tensor_add(s3.rearrange("p (k f) -> p k f", k=3),
                 g9[:, 0:9:3, :], g9[:, 1:9:3, :])
    V.tensor_add(o6[:, 0:3, :],
                 s3.rearrange("p (k f) -> p k f", k=3), g9[:, 2:9:3, :])
    nc.sync.dma_start(out=ov[:, 0:3, t0:t0 + f], in_=o6[:, 0:3, :])
    # diag: cov_ii = sum_j msq[3j+i]
    sd = wt("sd", 3)
    V.tensor_add(sd, msq[:, 0:3 * f], msq[:, 3 * f:6 * f])
    V.tensor_add(o6[:, 3:6, :],
                 sd.rearrange("p (k f) -> p k f", k=3), mq9[:, 6:9, :])
    nc.sync.dma_start(out=ov[:, 3:6, t0:t0 + f], in_=o6[:, 3:6, :])


def _pad_and_shard(quaternion, log_scale):
    n = quaternion.shape[0]
    ntot = N_CORES * NPC
    q16 = np.empty((ntot, 4), np.float16)
    l16 = np.empty((ntot, 3), np.float16)
    q16[:n] = quaternion.astype(np.float16)
    l16[:n] = log_scale.astype(np.float16)
    q16[n:] = np.array([1, 0, 0, 0], np.float16)
    l16[n:] = 0
    in_maps = []
    comp_order = (0, 3, 2, 1)   # (w, z, y, x) columns of quaternion
    for i in range(N_CORES):
        sl = slice(i * NPC, (i + 1) * NPC)
        qc = np.empty((P, 4, R), np.float16)
        for k, ci in enumerate(comp_order):
            qc[:, k, :] = np.ascontiguousarray(q16[sl, ci]).reshape(P, R)
        lc = np.empty((P, 3, R), np.float16)
        for ci in range(3):
            lc[:, ci, :] = np.ascontiguousarray(l16[sl, ci]).reshape(P, R)
        in_maps.append({"q_cat": qc, "l_cat": lc})
    return in_maps


def kernel_with_stats(quaternion, log_scale, trace=False):
    quaternion = np.asarray(quaternion, dtype=np.float32)
    log_scale = np.asarray(log_scale, dtype=np.float32)
    n = quaternion.shape[0]
    nc = _build()
    in_maps = _pad_and_shard(quaternion, log_scale)
    res = run_bass_kernel_spmd(nc, in_maps, core_ids=list(range(N_CORES)), trace=trace)
    out = np.empty((n, 3, 3), np.float32)
    # o_cat slots: [c01 c02 c12 | c00 c11 c22]
    slots = {0: [(0, 1), (1, 0)], 1: [(0, 2), (2, 0)], 2: [(1, 2), (2, 1)],
             3: [(0, 0)], 4: [(1, 1)], 5: [(2, 2)]}
    for s, ps in slots.items():
        full = np.concatenate([r["o_cat"][:, s, :].reshape(-1) for r in res.results])[:n]
        full = full.astype(np.float32)
        for (i, k) in ps:
            out[:, i, k] = full
    return out, res


def kernel(quaternion, log_scale):
    out, _ = kernel_with_stats(quaternion, log_scale, trace=False)
    return out
